# revision 5
# baseline (speedup 1.0000x reference)
"""Multi-head self-attention (B=2, S=2048, H=1024, 16 heads) on 8 trn2 cores.

Sharding: tensor-parallel over heads. Core c owns heads {2c, 2c+1}, i.e. a
contiguous 128-row slice of Wq/Wk/Wv and a contiguous 128-column slice of the
output features. Every core receives the full (transposed) hidden states and
computes its slice of the context; the host concatenates slices.

Per-core pipeline (matmuls run as float32r at full PE rate, ~1.6e-4 relmax
per matmul measured on HW):
  1. QT/KT/VT = W_cT.T @ hsT in [feat, seq] layout, N=512 matmuls, K=1024
     accumulated in PSUM over 8 k-tiles. Biases added on the PSUM->SBUF copy.
  2. VT tiles are PE-transposed into V_aug [k, 2, 65]: 64 V dims per head plus
     a ones column (the ones column makes the PV matmul also produce the
     softmax denominator for free).
  3. Attention per (batch, head, q-chunk of 1024): scores are computed
     TRANSPOSED, [k partitions, q free], so exp's mask bias is per-partition
     and no transpose of the (huge) probability matrix is ever needed.
     exp(0.125*s + maskbias) runs on ACT over [128, 1024] PSUM tiles.
     ctx_T[d, q] accumulates over the 16 k-tiles in PSUM with M=65 (64 dims +
     denominator row).
  4. Epilogue: ctx_T 128-col tiles are PE-transposed to [q, 65]; the
     denominator column is reciprocal'd (DVE) and applied as a per-partition
     scalar multiply. Output lands in natural [seq, feat] layout.
"""

import sys

if "/opt/trn_rl_repo" not in sys.path:
    sys.path.insert(0, "/opt/trn_rl_repo")

import numpy as np

import concourse.bass as bass
import concourse.mybir as mybir
import concourse.tile as tile
from concourse.bass_utils import run_bass_kernel_spmd
from concourse.masks import make_identity
from concourse.vector_clock import ScopedClock, VectorClock

B, S, H = 2, 2048, 1024
NH, HD = 16, 64
NCORES = 8
HPC = NH // NCORES          # heads per core = 2
F = HPC * HD                # output features per core = 128
BS = B * S                  # 4096
KT = H // 128               # k-tiles over the H contraction = 8
ST = S // 128               # 128-row seq tiles per batch = 16
QC = 1024                   # q-chunk for the attention phase
SC = 2048                   # seq-chunk for the QKV phase (== S, one batch)

F32 = mybir.dt.float32
F32R = mybir.dt.float32r


# ---------------------------------------------------------------------------
# Workarounds for this walrus build, which accepts at most ONE sync-wait per
# instruction while Tile's sem assignment happily emits several.
# ---------------------------------------------------------------------------

def _split_drain_and_barrier(self, tick_clock, wait_clock):
    """Replacement for TileContext._drain_and_barrier: put the tail drain's
    per-processor waits on one nofuse nop each instead of stacking them all
    on the single Drain ctrl instruction."""
    nc = self.nc
    gclock = tick_clock.global_clock
    n = len(gclock)
    for i in range(n):
        t = gclock[i]
        if t <= 0:
            continue
        nop = nc.sync.nop(nofuse=True, hint=f"tail_wait_p{i}")
        vec = [0] * n
        vec[i] = t
        wait_clock.add_sem_waits(nop.ins, ScopedClock({None: VectorClock(vec)}))
    nc.sync.drain()
    nc.all_engine_barrier()
    popped = nc._tile_sem_poison_stack.pop()
    assert popped is self._sem_poison
    nc.clear_and_free_semaphores(list(self.sems.allocated().values()))
    nc.all_engine_barrier()


tile.TileContext._drain_and_barrier = _split_drain_and_barrier


def split_excess_waits(nc, limit=1):
    """Move excess sync-waits onto same-engine NoOps inserted immediately
    before the instruction (engine queues are in-order, so a preceding nop
    wait is equivalent to a wait on the instruction itself)."""
    n_split = 0
    for f in nc.m.functions:
        for blk in f.blocks:
            insts = blk.instructions
            if not any(i.sync_info and len(i.sync_info.on_wait) > limit
                       for i in insts):
                continue
            new = []
            for inst in insts:
                si = inst.sync_info
                if si is not None and len(si.on_wait) > limit:
                    waits = list(si.on_wait)
                    extra, keep = waits[:-limit], waits[-limit:]
                    for k, w in enumerate(extra):
                        nop = mybir.InstNoOp(
                            name=f"{inst.name}-xw{k}",
                            sync_info=mybir.SyncInfo(on_wait=[w],
                                                     on_update=[]),
                        )
                        nop.engine = inst.engine
                        nc.register_instruction(nop)
                        new.append(nop)
                        n_split += 1
                    inst.sync_info = mybir.SyncInfo(
                        on_wait=keep, on_update=list(si.on_update))
                new.append(inst)
            blk.instructions = new
    return n_split


# ---------------------------------------------------------------------------
# Kernel build
# ---------------------------------------------------------------------------

def build_nc():
    nc = bass.Bass("TRN2", target_bir_lowering=False, debug=False,
                   num_devices=NCORES)

    hsT = nc.dram_tensor("hsT", [H, BS], F32R, kind="ExternalInput").ap()
    wqT = nc.dram_tensor("wqT", [H, F], F32R, kind="ExternalInput").ap()
    wkT = nc.dram_tensor("wkT", [H, F], F32R, kind="ExternalInput").ap()
    wvT = nc.dram_tensor("wvT", [H, F], F32R, kind="ExternalInput").ap()
    bq = nc.dram_tensor("bq", [F], F32, kind="ExternalInput").ap()
    bk = nc.dram_tensor("bk", [F], F32, kind="ExternalInput").ap()
    bv = nc.dram_tensor("bv", [F], F32, kind="ExternalInput").ap()
    mask = nc.dram_tensor("mask", [B, S], F32, kind="ExternalInput").ap()
    out = nc.dram_tensor("out", [BS, F], F32, kind="ExternalOutput").ap()

    with tile.TileContext(nc) as tc:
        with (
            tc.tile_pool(name="singles", bufs=1) as singles,
            tc.tile_pool(name="qk_sb", bufs=1) as qk_sb,
        ):
            # ---- constants -------------------------------------------------
            ident = singles.tile([128, 128], F32)
            make_identity(nc, ident)

            w_sb = {}
            for nm, dram in (("q", wqT), ("k", wkT), ("v", wvT)):
                t = singles.tile([128, KT, F], F32R, tag=f"w{nm}")
                nc.sync.dma_start(
                    out=t, in_=dram.rearrange("(kt p) m -> p kt m", p=128))
                w_sb[nm] = t

            b_sb = {}
            for nm, dram in (("q", bq), ("k", bk), ("v", bv)):
                t = singles.tile([128, 1], F32, tag=f"b{nm}")
                nc.sync.dma_start(
                    out=t, in_=dram.rearrange("(p one) -> p one", one=1))
                b_sb[nm] = t

            # mask -> additive bias per key position: (m - 1) * 10000
            maskT = singles.tile([128, B, ST], F32)
            nc.sync.dma_start(
                out=maskT, in_=mask.rearrange("b (t p) -> p b t", p=128))
            mbias = singles.tile([128, B, ST], F32)
            nc.vector.tensor_scalar(
                out=mbias, in0=maskT, scalar1=10000.0, scalar2=-10000.0,
                op0=mybir.AluOpType.mult, op1=mybir.AluOpType.add)
            # warm the exp table set while DMAs stream in
            dummy = singles.tile([128, 1], F32)
            nc.scalar.activation(dummy, b_sb["q"],
                                 mybir.ActivationFunctionType.Exp)

            # ---- long-lived activations -----------------------------------
            qT = qk_sb.tile([128, BS], F32R, tag="qT")    # [feat, seq]
            kT = qk_sb.tile([128, BS], F32R, tag="kT")
            # V with appended ones column, per (b, seq-tile, head):
            # [k-within-tile, b*ST+t, head, 65]
            v_aug = qk_sb.tile([128, B * ST, HPC, HD + 1], F32R, tag="vaug")
            # fill the ones column via a DVE copy (memset can't write f32r,
            # and the f32r matmul verifier wants a rounding producer)
            ones_t = singles.tile([128, 1], F32)
            nc.vector.memset(ones_t, 1.0)
            for bt in range(B * ST):
                nc.vector.tensor_copy(
                    out=v_aug[:, bt, :, HD:HD + 1],
                    in_=ones_t.to_broadcast((128, HPC, 1)))

            # ---- phase 1: QKV projections ---------------------------------
            with (
                tc.tile_pool(name="hs_sb", bufs=1) as hs_pool,
                tc.tile_pool(name="vt_sb", bufs=1) as vt_pool,
                tc.tile_pool(name="qkv_ps", bufs=2, space="PSUM") as proj_ps,
                tc.tile_pool(name="vtr_ps", bufs=2, space="PSUM") as vtr_ps,
            ):
                vT = vt_pool.tile([128, BS], F32)
                for chunk in range(BS // SC):
                    c0 = chunk * SC
                    hs_t = [hs_pool.tile([128, SC], F32R, tag=f"hs{kt}",
                                         name=f"hs{kt}")
                            for kt in range(KT)]
                    for kt in range(KT):
                        nc.sync.dma_start(
                            out=hs_t[kt],
                            in_=hsT[kt * 128:(kt + 1) * 128, c0:c0 + SC])
                    for sb in range(SC // 512):
                        s0 = sb * 512
                        for nm, dst in (("q", qT), ("k", kT), ("v", vT)):
                            ps = proj_ps.tile([128, 512], F32, tag=f"p{nm}")
                            for kt in range(KT):
                                nc.tensor.matmul(
                                    ps,
                                    w_sb[nm][:, kt, :],
                                    hs_t[kt][:, s0:s0 + 512],
                                    start=(kt == 0), stop=(kt == KT - 1))
                            nc.vector.tensor_scalar_add(
                                out=dst[:, c0 + s0:c0 + s0 + 512],
                                in0=ps, scalar1=b_sb[nm])

                # ---- phase 1b: transpose V into [seq, feat] + ones --------
                for bt in range(B * ST):
                    ps = vtr_ps.tile([128, 128], F32, tag="vt")
                    nc.tensor.transpose(
                        ps, vT[:, bt * 128:(bt + 1) * 128], ident)
                    for h in range(HPC):
                        nc.vector.tensor_copy(
                            out=v_aug[:, bt, h, 0:HD],
                            in_=ps[:, h * HD:(h + 1) * HD])

            # ---- phase 2: attention ---------------------------------------
            with (
                tc.tile_pool(name="attn_sb", bufs=3) as attn_sb,
                tc.tile_pool(name="eps_sb", bufs=3) as eps_pool,
                tc.tile_pool(name="out_sb", bufs=2) as out_pool,
                tc.tile_pool(name="sc_ps", bufs=2, space="PSUM") as sc_ps,
                tc.tile_pool(name="cx_ps", bufs=2, space="PSUM") as cx_ps,
            ):
                out_r = out.rearrange("(n p) j -> p n j", p=128)
                for b in range(B):
                    for h in range(HPC):
                        hs0 = h * HD
                        for qc in range(S // QC):
                            q0 = b * S + qc * QC
                            ctx = cx_ps.tile([HD + 1, QC], F32, tag="cx")
                            for kt in range(ST):
                                k0 = b * S + kt * 128
                                sc = sc_ps.tile([128, QC], F32, tag="sc")
                                for qq in range(QC // 512):
                                    nc.tensor.matmul(
                                        sc[:, qq * 512:(qq + 1) * 512],
                                        kT[hs0:hs0 + HD, k0:k0 + 128],
                                        qT[hs0:hs0 + HD,
                                           q0 + qq * 512:q0 + (qq + 1) * 512],
                                        start=True, stop=True)
                                es = eps_pool.tile([128, QC], F32R, tag="es")
                                nc.scalar.activation(
                                    es, sc, mybir.ActivationFunctionType.Exp,
                                    bias=mbias[:, b, kt:kt + 1], scale=0.125)
                                for qq in range(QC // 512):
                                    nc.tensor.matmul(
                                        ctx[:, qq * 512:(qq + 1) * 512],
                                        v_aug[:, b * ST + kt, h, :],
                                        es[:, qq * 512:(qq + 1) * 512],
                                        start=(kt == 0), stop=(kt == ST - 1))
                            # epilogue: normalize + transpose to [q, d]
                            ctx_sb = attn_sb.tile([HD + 1, QC], F32, tag="cxs")
                            nc.vector.tensor_copy(out=ctx_sb, in_=ctx)
                            ot = out_pool.tile([128, QC // 128, HD], F32,
                                               tag="ot")
                            for qt in range(QC // 128):
                                tr = cx_ps.tile([128, HD + 1], F32, tag="cx")
                                nc.tensor.transpose(
                                    tr, ctx_sb[:, qt * 128:(qt + 1) * 128],
                                    ident[0:HD + 1, 0:HD + 1])
                                rc = attn_sb.tile([128, 1], F32, tag="rc")
                                nc.vector.reciprocal(rc, tr[:, HD:HD + 1])
                                nc.vector.tensor_scalar_mul(
                                    out=ot[:, qt, :], in0=tr[:, 0:HD],
                                    scalar1=rc)
                            nc.sync.dma_start(
                                out=out_r[:, q0 // 128:q0 // 128 + QC // 128,
                                          hs0:hs0 + HD],
                                in_=ot)

    split_excess_waits(nc)
    return nc


_NC_CACHE = None


def _get_nc():
    global _NC_CACHE
    if _NC_CACHE is None:
        _NC_CACHE = build_nc()
    return _NC_CACHE


def make_in_maps(hidden_states, attention_mask, Wq, bq, Wk, bk, Wv, bv):
    hs = np.asarray(hidden_states, np.float32).reshape(BS, H)
    hsT = np.ascontiguousarray(hs.T)
    mask = np.asarray(attention_mask, np.float32)
    in_maps = []
    for c in range(NCORES):
        sl = slice(c * F, (c + 1) * F)
        in_maps.append({
            "hsT": hsT,
            "wqT": np.ascontiguousarray(np.asarray(Wq, np.float32)[sl].T),
            "wkT": np.ascontiguousarray(np.asarray(Wk, np.float32)[sl].T),
            "wvT": np.ascontiguousarray(np.asarray(Wv, np.float32)[sl].T),
            "bq": np.asarray(bq, np.float32)[sl],
            "bk": np.asarray(bk, np.float32)[sl],
            "bv": np.asarray(bv, np.float32)[sl],
            "mask": mask,
        })
    return in_maps


def run(in_maps, trace=False):
    nc = _get_nc()
    return run_bass_kernel_spmd(nc, in_maps, list(range(NCORES)), trace=trace)


def assemble(results):
    full = np.empty((B, S, H), np.float32)
    for c in range(NCORES):
        full[:, :, c * F:(c + 1) * F] = results[c]["out"].reshape(B, S, F)
    return full


def kernel(hidden_states, attention_mask, Wq, bq, Wk, bk, Wv, bv):
    res = run(make_in_maps(hidden_states, attention_mask,
                           Wq, bq, Wk, bk, Wv, bv))
    return assemble(res.results)


# revision 9
# speedup vs baseline: 1.2815x; 1.2815x over previous
"""Multi-head self-attention (B=2, S=2048, H=1024, 16 heads) on 8 trn2 cores.

Sharding: tensor-parallel over heads. Core c owns heads {2c, 2c+1}, i.e. a
contiguous 128-row slice of Wq/Wk/Wv and a contiguous 128-column slice of the
output features. Every core receives the full (transposed) hidden states and
computes its slice of the context; the host concatenates slices.

Per-core pipeline (matmuls run as float32r at full PE rate, ~1.6e-4 relmax
per matmul measured on HW):
  1. QT/KT/VT = W_cT.T @ hsT in [feat, seq] layout, N=512 matmuls, K=1024
     accumulated in PSUM over 8 k-tiles. Biases added on the PSUM->SBUF copy.
  2. VT tiles are PE-transposed into V_aug [k, 2, 65]: 64 V dims per head plus
     a ones column (the ones column makes the PV matmul also produce the
     softmax denominator for free).
  3. Attention per (batch, head, q-chunk of 1024): scores are computed
     TRANSPOSED, [k partitions, q free], so exp's mask bias is per-partition
     and no transpose of the (huge) probability matrix is ever needed.
     exp(0.125*s + maskbias) runs on ACT over [128, 1024] PSUM tiles.
     ctx_T[d, q] accumulates over the 16 k-tiles in PSUM with M=65 (64 dims +
     denominator row).
  4. Epilogue: ctx_T 128-col tiles are PE-transposed to [q, 65]; the
     denominator column is reciprocal'd (DVE) and applied as a per-partition
     scalar multiply. Output lands in natural [seq, feat] layout.
"""

import sys

if "/opt/trn_rl_repo" not in sys.path:
    sys.path.insert(0, "/opt/trn_rl_repo")

import numpy as np

import concourse.bass as bass
import concourse.mybir as mybir
import concourse.tile as tile
from concourse.bass_utils import run_bass_kernel_spmd
from concourse.masks import make_identity
from concourse.vector_clock import ScopedClock, VectorClock

B, S, H = 2, 2048, 1024
NH, HD = 16, 64
NCORES = 8
HPC = NH // NCORES          # heads per core = 2
F = HPC * HD                # output features per core = 128
BS = B * S                  # 4096
KT = H // 128               # k-tiles over the H contraction = 8
ST = S // 128               # 128-row seq tiles per batch = 16
QC = 1024                   # q-chunk for the attention phase
SC = 2048                   # seq-chunk for the QKV phase (== S, one batch)

F32 = mybir.dt.float32
F32R = mybir.dt.float32r
BF16 = mybir.dt.bfloat16


# ---------------------------------------------------------------------------
# Workarounds for this walrus build, which accepts at most ONE sync-wait per
# instruction while Tile's sem assignment happily emits several.
# ---------------------------------------------------------------------------

def _split_drain_and_barrier(self, tick_clock, wait_clock):
    """Replacement for TileContext._drain_and_barrier: put the tail drain's
    per-processor waits on one nofuse nop each instead of stacking them all
    on the single Drain ctrl instruction."""
    nc = self.nc
    gclock = tick_clock.global_clock
    n = len(gclock)
    for i in range(n):
        t = gclock[i]
        if t <= 0:
            continue
        nop = nc.sync.nop(nofuse=True, hint=f"tail_wait_p{i}")
        vec = [0] * n
        vec[i] = t
        wait_clock.add_sem_waits(nop.ins, ScopedClock({None: VectorClock(vec)}))
    nc.sync.drain()
    nc.all_engine_barrier()
    popped = nc._tile_sem_poison_stack.pop()
    assert popped is self._sem_poison
    nc.clear_and_free_semaphores(list(self.sems.allocated().values()))
    nc.all_engine_barrier()


tile.TileContext._drain_and_barrier = _split_drain_and_barrier


def split_excess_waits(nc, limit=1):
    """Move excess sync-waits onto same-engine NoOps inserted immediately
    before the instruction (engine queues are in-order, so a preceding nop
    wait is equivalent to a wait on the instruction itself)."""
    n_split = 0
    for f in nc.m.functions:
        for blk in f.blocks:
            insts = blk.instructions
            if not any(i.sync_info and len(i.sync_info.on_wait) > limit
                       for i in insts):
                continue
            new = []
            for inst in insts:
                si = inst.sync_info
                if si is not None and len(si.on_wait) > limit:
                    waits = list(si.on_wait)
                    extra, keep = waits[:-limit], waits[-limit:]
                    for k, w in enumerate(extra):
                        nop = mybir.InstNoOp(
                            name=f"{inst.name}-xw{k}",
                            sync_info=mybir.SyncInfo(on_wait=[w],
                                                     on_update=[]),
                        )
                        nop.engine = inst.engine
                        nc.register_instruction(nop)
                        new.append(nop)
                        n_split += 1
                    inst.sync_info = mybir.SyncInfo(
                        on_wait=keep, on_update=list(si.on_update))
                new.append(inst)
            blk.instructions = new
    return n_split


# ---------------------------------------------------------------------------
# Kernel build
# ---------------------------------------------------------------------------

def build_nc():
    nc = bass.Bass("TRN2", target_bir_lowering=False, debug=False,
                   num_devices=NCORES)

    hsT = nc.dram_tensor("hsT", [H, BS], F32R, kind="ExternalInput").ap()
    wqT = nc.dram_tensor("wqT", [H, F], F32R, kind="ExternalInput").ap()
    wkT = nc.dram_tensor("wkT", [H, F], F32R, kind="ExternalInput").ap()
    wvT = nc.dram_tensor("wvT", [H, F], F32R, kind="ExternalInput").ap()
    bq = nc.dram_tensor("bq", [F], F32, kind="ExternalInput").ap()
    bk = nc.dram_tensor("bk", [F], F32, kind="ExternalInput").ap()
    bv = nc.dram_tensor("bv", [F], F32, kind="ExternalInput").ap()
    mask = nc.dram_tensor("mask", [B, S], F32, kind="ExternalInput").ap()
    out = nc.dram_tensor("out", [BS, F], F32, kind="ExternalOutput").ap()

    with tile.TileContext(nc) as tc:
        with (
            tc.tile_pool(name="singles", bufs=1) as singles,
            tc.tile_pool(name="qk_sb", bufs=1) as qk_sb,
        ):
            # ---- constants -------------------------------------------------
            ident = singles.tile([128, 128], F32)
            make_identity(nc, ident)

            w_sb = {}
            for nm, dram in (("q", wqT), ("k", wkT), ("v", wvT)):
                t = singles.tile([128, KT, F], F32R, tag=f"w{nm}")
                nc.sync.dma_start(
                    out=t, in_=dram.rearrange("(kt p) m -> p kt m", p=128))
                w_sb[nm] = t

            b_sb = {}
            for nm, dram in (("q", bq), ("k", bk), ("v", bv)):
                t = singles.tile([128, 1], F32, tag=f"b{nm}")
                nc.sync.dma_start(
                    out=t, in_=dram.rearrange("(p one) -> p one", one=1))
                b_sb[nm] = t

            # mask -> additive bias per key position: (m - 1) * 10000
            maskT = singles.tile([128, B, ST], F32)
            nc.sync.dma_start(
                out=maskT, in_=mask.rearrange("b (t p) -> p b t", p=128))
            mbias = singles.tile([128, B, ST], F32)
            nc.vector.tensor_scalar(
                out=mbias, in0=maskT, scalar1=10000.0, scalar2=-10000.0,
                op0=mybir.AluOpType.mult, op1=mybir.AluOpType.add)
            # warm the exp table set while DMAs stream in
            dummy = singles.tile([128, 1], F32)
            nc.scalar.activation(dummy, b_sb["q"],
                                 mybir.ActivationFunctionType.Exp)

            # ---- long-lived activations -----------------------------------
            # Attention operands live in bf16: the PE streams bf16 at the
            # same cycles/row as f32r but with a separate (pipelined)
            # LDWEIGHTS instead of the serialized self-loading 4-byte form,
            # and the moving operand can be 1024 wide.
            qT = qk_sb.tile([128, BS], BF16, tag="qT")    # [feat, seq]
            kT = qk_sb.tile([128, BS], BF16, tag="kT")
            # V with appended ones column, per (b, seq-tile, head):
            # [k-within-tile, b*ST+t, head, 65]
            v_aug = qk_sb.tile([128, B * ST, HPC, HD + 1], BF16, tag="vaug")
            ones_t = singles.tile([128, 1], BF16)
            nc.vector.memset(ones_t, 1.0)
            for bt in range(B * ST):
                nc.vector.tensor_copy(
                    out=v_aug[:, bt, :, HD:HD + 1],
                    in_=ones_t.to_broadcast((128, HPC, 1)))

            # ---- phase 1: QKV projections ---------------------------------
            with (
                tc.tile_pool(name="hs_sb", bufs=1) as hs_pool,
                tc.tile_pool(name="vt_sb", bufs=1) as vt_pool,
                tc.tile_pool(name="qkv_ps", bufs=2, space="PSUM") as proj_ps,
                tc.tile_pool(name="vtr_ps", bufs=2, space="PSUM") as vtr_ps,
            ):
                vT = vt_pool.tile([128, BS], F32)
                for chunk in range(BS // SC):
                    c0 = chunk * SC
                    hs_t = [hs_pool.tile([128, SC], F32R, tag=f"hs{kt}",
                                         name=f"hs{kt}")
                            for kt in range(KT)]
                    for kt in range(KT):
                        nc.sync.dma_start(
                            out=hs_t[kt],
                            in_=hsT[kt * 128:(kt + 1) * 128, c0:c0 + SC])
                    for sb in range(SC // 512):
                        s0 = sb * 512
                        for nm, dst in (("q", qT), ("k", kT), ("v", vT)):
                            ps = proj_ps.tile([128, 512], F32, tag=f"p{nm}")
                            for kt in range(KT):
                                nc.tensor.matmul(
                                    ps,
                                    w_sb[nm][:, kt, :],
                                    hs_t[kt][:, s0:s0 + 512],
                                    start=(kt == 0), stop=(kt == KT - 1))
                            nc.vector.tensor_scalar_add(
                                out=dst[:, c0 + s0:c0 + s0 + 512],
                                in0=ps, scalar1=b_sb[nm])

                # ---- phase 1b: transpose V into [seq, feat] + ones --------
                for bt in range(B * ST):
                    ps = vtr_ps.tile([128, 128], F32, tag="vt")
                    nc.tensor.transpose(
                        ps, vT[:, bt * 128:(bt + 1) * 128], ident)
                    for h in range(HPC):
                        nc.vector.tensor_copy(
                            out=v_aug[:, bt, h, 0:HD],
                            in_=ps[:, h * HD:(h + 1) * HD])

            # ---- phase 2: attention ---------------------------------------
            with (
                tc.tile_pool(name="attn_sb", bufs=3) as attn_sb,
                tc.tile_pool(name="eps_sb", bufs=3) as eps_pool,
                tc.tile_pool(name="out_sb", bufs=2) as out_pool,
                tc.tile_pool(name="sc_ps", bufs=2, space="PSUM") as sc_ps,
                tc.tile_pool(name="cx_ps", bufs=2, space="PSUM") as cx_ps,
            ):
                out_r = out.rearrange("(n p) j -> p n j", p=128)
                for b in range(B):
                    for h in range(HPC):
                        hs0 = h * HD
                        for qc in range(S // QC):
                            q0 = b * S + qc * QC
                            ctx = cx_ps.tile([HD + 1, QC], F32, tag="cx")
                            for kt in range(ST):
                                k0 = b * S + kt * 128
                                sc = sc_ps.tile([128, QC], F32, tag="sc")
                                for qq in range(QC // 512):
                                    nc.tensor.matmul(
                                        sc[:, qq * 512:(qq + 1) * 512],
                                        kT[hs0:hs0 + HD, k0:k0 + 128],
                                        qT[hs0:hs0 + HD,
                                           q0 + qq * 512:q0 + (qq + 1) * 512],
                                        start=True, stop=True)
                                es = eps_pool.tile([128, QC], BF16, tag="es")
                                nc.scalar.activation(
                                    es, sc, mybir.ActivationFunctionType.Exp,
                                    bias=mbias[:, b, kt:kt + 1], scale=0.125)
                                for qq in range(QC // 512):
                                    nc.tensor.matmul(
                                        ctx[:, qq * 512:(qq + 1) * 512],
                                        v_aug[:, b * ST + kt, h, :],
                                        es[:, qq * 512:(qq + 1) * 512],
                                        start=(kt == 0), stop=(kt == ST - 1))
                            # epilogue: normalize + transpose to [q, d]
                            ctx_sb = attn_sb.tile([HD + 1, QC], F32, tag="cxs")
                            nc.vector.tensor_copy(out=ctx_sb, in_=ctx)
                            ot = out_pool.tile([128, QC // 128, HD], F32,
                                               tag="ot")
                            for qt in range(QC // 128):
                                tr = cx_ps.tile([128, HD + 1], F32, tag="cx")
                                nc.tensor.transpose(
                                    tr, ctx_sb[:, qt * 128:(qt + 1) * 128],
                                    ident[0:HD + 1, 0:HD + 1])
                                rc = attn_sb.tile([128, 1], F32, tag="rc")
                                nc.vector.reciprocal(rc, tr[:, HD:HD + 1])
                                nc.vector.tensor_scalar_mul(
                                    out=ot[:, qt, :], in0=tr[:, 0:HD],
                                    scalar1=rc)
                            nc.sync.dma_start(
                                out=out_r[:, q0 // 128:q0 // 128 + QC // 128,
                                          hs0:hs0 + HD],
                                in_=ot)

    split_excess_waits(nc)
    return nc


_NC_CACHE = None


def _get_nc():
    global _NC_CACHE
    if _NC_CACHE is None:
        _NC_CACHE = build_nc()
    return _NC_CACHE


def make_in_maps(hidden_states, attention_mask, Wq, bq, Wk, bk, Wv, bv):
    hs = np.asarray(hidden_states, np.float32).reshape(BS, H)
    hsT = np.ascontiguousarray(hs.T)
    mask = np.asarray(attention_mask, np.float32)
    in_maps = []
    for c in range(NCORES):
        sl = slice(c * F, (c + 1) * F)
        in_maps.append({
            "hsT": hsT,
            "wqT": np.ascontiguousarray(np.asarray(Wq, np.float32)[sl].T),
            "wkT": np.ascontiguousarray(np.asarray(Wk, np.float32)[sl].T),
            "wvT": np.ascontiguousarray(np.asarray(Wv, np.float32)[sl].T),
            "bq": np.asarray(bq, np.float32)[sl],
            "bk": np.asarray(bk, np.float32)[sl],
            "bv": np.asarray(bv, np.float32)[sl],
            "mask": mask,
        })
    return in_maps


def run(in_maps, trace=False):
    nc = _get_nc()
    return run_bass_kernel_spmd(nc, in_maps, list(range(NCORES)), trace=trace)


def assemble(results):
    full = np.empty((B, S, H), np.float32)
    for c in range(NCORES):
        full[:, :, c * F:(c + 1) * F] = results[c]["out"].reshape(B, S, F)
    return full


def kernel(hidden_states, attention_mask, Wq, bq, Wk, bk, Wv, bv):
    res = run(make_in_maps(hidden_states, attention_mask,
                           Wq, bq, Wk, bk, Wv, bv))
    return assemble(res.results)
